# revision 21
# baseline (speedup 1.0000x reference)
"""Trainium2 Bass kernel for nn_BATransform — PE-matmul spatial mix.

Per core (one sample, c-major [C, 8192] f32 in DRAM):
  load+convert+shuffle x -> L-layout (u,v,t,a,b) bf16 chunks
  conv1(1x1)+BN+ReLU -> o (k=8) in L-layout -> pools -> p/q/tm softmax
  build per-group 128x128 mix matrices M_g = TM (x) P (x) Q on DVE
  T-in: PE transposes -> XT[(t,a,b), (u,v,c)]
  mix: 8 matmuls/chunk with M_g -> YT -> T-out: PE transposes -> y2L
  unshuffle (2-pass) -> y2 natural c-major -> conv2+BN+ReLU -> store
"""

import numpy as np
import ml_dtypes

NB, C, T, H, W = 8, 512, 8, 32, 32
S, TS, K, TK = 4, 2, 8, 8
EPS = 1e-5
POS = T * H * W        # 8192
NCORES = 8
NCHUNK = 4
CPK = 128

_CACHE = {}


def _split_waits(nc, mybir, max_waits=1):
    """Walrus accepts at most one sync-wait per instruction; hoist extras
    onto same-engine NoOps placed just before."""
    for f in nc.m.functions:
        for bb in f.blocks:
            out = []
            for inst in bb.instructions:
                si = inst.sync_info
                if si is not None and si.on_wait and len(si.on_wait) > max_waits:
                    waits = list(si.on_wait)
                    keep = waits[-max_waits:]
                    extra = waits[:-max_waits]
                    k = 0
                    while extra:
                        chunk, extra = extra[:max_waits], extra[max_waits:]
                        nop = mybir.InstNoOp(
                            name=f"{inst.name}-ws{k}",
                            sync_info=mybir.SyncInfo(on_wait=chunk, on_update=[]),
                        )
                        nop.engine = inst.engine
                        nc.register_instruction(nop)
                        out.append(nop)
                        k += 1
                    inst.sync_info = mybir.SyncInfo(
                        on_wait=keep, on_update=list(si.on_update)
                    )
                out.append(inst)
            bb.instructions = out


def _build():
    import concourse.bass as bass
    import concourse.tile as tile
    import concourse.mybir as mybir
    from concourse.masks import make_identity

    f32 = mybir.dt.float32
    bf16 = mybir.dt.bfloat16
    AF = mybir.ActivationFunctionType
    ALU = mybir.AluOpType
    AX = mybir.AxisListType

    nc = bass.Bass("TRN2", target_bir_lowering=False, debug=False,
                   num_devices=NCORES)

    x_d = nc.dram_tensor("x", [C, POS], f32, kind="ExternalInput")
    w1t_d = nc.dram_tensor("w1t", [C, K], bf16, kind="ExternalInput")
    b1_d = nc.dram_tensor("b1", [K, 1], f32, kind="ExternalInput")
    wpt_d = nc.dram_tensor("wpt", [K, 4 * 128], f32, kind="ExternalInput")
    bp_d = nc.dram_tensor("bp", [T, 128], f32, kind="ExternalInput")
    wqt_d = nc.dram_tensor("wqt", [K, 4 * 128], f32, kind="ExternalInput")
    bq_d = nc.dram_tensor("bq", [T, 128], f32, kind="ExternalInput")
    wtt_d = nc.dram_tensor("wtt", [K, 2 * 32], f32, kind="ExternalInput")
    bt_d = nc.dram_tensor("bt", [1, 32], f32, kind="ExternalInput")
    w2t_d = nc.dram_tensor("w2t", [C, C], bf16, kind="ExternalInput")
    b2_d = nc.dram_tensor("b2", [C, 1], f32, kind="ExternalInput")
    mask_d = nc.dram_tensor("mask", [128, 128], f32, kind="ExternalInput")
    y_d = nc.dram_tensor("y", [C, POS], f32, kind="ExternalOutput")

    with tile.TileContext(nc) as tc:
        with (
            tc.tile_pool(name="consts", bufs=1) as consts,
            tc.tile_pool(name="stage", bufs=4) as stage,      # f32 [128,2048]
            tc.tile_pool(name="short", bufs=4) as shortp,     # bf16 [128,8192]
            tc.tile_pool(name="long", bufs=4) as longp,       # XT + y2n shared
            tc.tile_pool(name="osb", bufs=1) as osbp,
            tc.tile_pool(name="sm2", bufs=1) as sm2,
            tc.tile_pool(name="mpool", bufs=1) as mpool,
            tc.tile_pool(name="outp", bufs=6) as outp,        # f32 [128,512]
            tc.tile_pool(name="psA", bufs=1, space="PSUM") as psa,
            tc.tile_pool(name="psB", bufs=4, space="PSUM") as psp,
            tc.tile_pool(name="dram", bufs=1, space="DRAM") as dramp,
        ):
            # ---- weights / consts ----
            w1t = []
            for q in range(NCHUNK):
                t_ = consts.tile([CPK, K], bf16, tag=f"w1t{q}")
                nc.sync.dma_start(t_, w1t_d[:][q * CPK:(q + 1) * CPK, :])
                w1t.append(t_)
            b1 = consts.tile([K, 1], f32, tag="b1")
            nc.sync.dma_start(b1, b1_d[:])
            wpt = consts.tile([K, 4 * 128], f32, tag="wpt")
            nc.sync.dma_start(wpt, wpt_d[:])
            bp = consts.tile([T, 128], f32, tag="bp")
            nc.sync.dma_start(bp, bp_d[:])
            wqt = consts.tile([K, 4 * 128], f32, tag="wqt")
            nc.sync.dma_start(wqt, wqt_d[:])
            bq = consts.tile([T, 128], f32, tag="bq")
            nc.sync.dma_start(bq, bq_d[:])
            wtt = consts.tile([K, 2 * 32], f32, tag="wtt")
            nc.sync.dma_start(wtt, wtt_d[:])
            bt = consts.tile([1, 32], f32, tag="bt")
            nc.sync.dma_start(bt, bt_d[:])
            w2t = []
            for q in range(NCHUNK):
                t_ = consts.tile([CPK, C], bf16, tag=f"w2t{q}")
                nc.sync.dma_start(t_, w2t_d[:][q * CPK:(q + 1) * CPK, :])
                w2t.append(t_)
            b2 = []
            for oc in range(NCHUNK):
                t_ = consts.tile([CPK, 1], f32, tag=f"b2_{oc}")
                nc.sync.dma_start(t_, b2_d[:][oc * CPK:(oc + 1) * CPK, :])
                b2.append(t_)
            mask = consts.tile([128, 128], f32, tag="mask")
            nc.sync.dma_start(mask, mask_d[:])
            ident = consts.tile([128, 128], bf16, tag="ident")
            make_identity(nc, ident)

            o_sb = osbp.tile([K, POS], bf16, tag="o")
            # conv1 accumulators: 4 psum banks x 4 partition offsets (32j)
            c1ps = [psa.tile([128, 512], f32, tag=f"c1ps_{i}",
                             name=f"c1ps_{i}")
                    for i in range(4)]

            xt = []     # XT chunks [tab, (o,c)]
            # ================= load + shuffle + conv1-partial + T-in ======
            for ch in range(NCHUNK):
                # fused cast+Lshuf: natural f32 quarter -> L3 bf16 quarter
                # iter (ta 8, bv 32, u 8): in strides (256, 1, 32)
                x2 = shortp.tile([CPK, POS], bf16, tag="sh", name=f"x2_{ch}")
                for q in range(4):
                    xf = stage.tile([CPK, POS // 4], f32, tag="stage")
                    nc.sync.dma_start(
                        xf, x_d[:][ch * CPK:(ch + 1) * CPK,
                                   q * 2048:(q + 1) * 2048])
                    in_ap = bass.AP(tensor=xf.tensor, offset=xf.offset,
                                    ap=[list(xf.ap[0]), [256, 8], [1, 32],
                                        [32, 8]])
                    if q % 2 == 0:
                        nc.scalar.activation(
                            x2[:, q * 2048:(q + 1) * 2048], in_ap, AF.Copy)
                    else:
                        nc.vector.tensor_copy(
                            x2[:, q * 2048:(q + 1) * 2048], in_ap)

                # conv1 partials: psum[nt] += w1t[ch].T @ x2[:, nt*512...]
                for nt in range(16):
                    ps = c1ps[nt % 4][(nt // 4) * 32:(nt // 4) * 32 + K, :]
                    nc.tensor.matmul(
                        ps, w1t[ch], x2[:, nt * 512:(nt + 1) * 512],
                        start=(ch == 0), stop=(ch == NCHUNK - 1),
                        tile_position=(0, (nt // 4) * 32))

                # T-in: per uv-quad: 4 transposes + 1 copy
                xt_ch = longp.tile([128, POS], bf16, tag="lg",
                                   name=f"xt_{ch}")
                for oct in range(8):
                    tps = psp.tile([128, 1024], bf16, tag="ps")
                    for j in range(8):
                        o = oct * 8 + j
                        in_ap = bass.AP(tensor=x2.tensor,
                                        offset=x2.offset + o,
                                        ap=[list(x2.ap[0]), [64, 128]])
                        nc.tensor.transpose(
                            tps[:, j * 128:(j + 1) * 128], in_ap, ident)
                    nc.vector.tensor_copy(
                        xt_ch[:, oct * 1024:(oct + 1) * 1024], tps)
                xt.append(xt_ch)

            # ---- conv1 finalize: ReLU(bias) -> o_sb; block sums -> tpp --
            tpp = sm2.tile([K, 16], f32, tag="tpp")
            for nt in range(16):
                ps = c1ps[nt % 4][(nt // 4) * 32:(nt // 4) * 32 + K, :]
                nc.scalar.activation(
                    o_sb[:, nt * 512:(nt + 1) * 512], ps, AF.Relu,
                    bias=b1, scale=1.0, accum_out=tpp[:, nt:nt + 1])

            # ---- pools (o in L3 layout (t,a,b,v,u)) ----
            # rp[k, (t,a)] = max over (b, v, u) = inner 256
            rp = sm2.tile([K, T * S], f32, tag="rp")
            nc.vector.tensor_reduce(
                rp, bass.AP(tensor=o_sb.tensor, offset=o_sb.offset,
                            ap=[list(o_sb.ap[0]), [256, 32], [1, 256]]),
                AX.X, ALU.max)
            # cp[k, (t,b)] = max over (a, (v,u) 64)
            cp = sm2.tile([K, T * S], f32, tag="cp")
            nc.vector.tensor_reduce(
                cp, bass.AP(tensor=o_sb.tensor, offset=o_sb.offset,
                            ap=[list(o_sb.ap[0]), [1024, 8], [64, 4],
                                [256, 4], [1, 64]]),
                AX.XY, ALU.max)
            # tp[k, ts] = sum of tpp block sums
            tp = sm2.tile([K, TS], f32, tag="tp")
            nc.vector.tensor_reduce(
                tp, tpp.rearrange("p (s j) -> p s j", j=8), AX.X, ALU.add)

            # ---- p/q/tm small matmuls + softmax ----
            def softmax_rowgroups(name, ps_tile, bias_tile, parts, ngroups,
                                  gsz):
                raw = sm2.tile([parts, ngroups * gsz], f32, tag=f"{name}_raw")
                nc.vector.tensor_tensor(raw, ps_tile, bias_tile, ALU.add)
                ex = sm2.tile([parts, ngroups * gsz], f32, tag=f"{name}_ex")
                nc.scalar.activation(ex, raw, AF.Exp)
                ss = sm2.tile([parts, ngroups], f32, tag=f"{name}_ss")
                nc.vector.tensor_reduce(
                    ss, ex.rearrange("p (g r) -> p g r", r=gsz), AX.X,
                    ALU.add)
                rr = sm2.tile([parts, ngroups], f32, tag=f"{name}_rr")
                nc.vector.reciprocal(rr, ss)
                sm = sm2.tile([parts, ngroups * gsz], f32, tag=f"{name}_sm")
                for b in range(gsz):
                    if name in ("p", "q"):
                        # write col g*16+a*4+b at perm pos b*32+g*4+a
                        dst = bass.AP(
                            tensor=sm.tensor, offset=sm.offset + b * 32,
                            ap=[list(sm.ap[0]), [4, 8], [1, 4]])
                    else:
                        # t: col g*4+a2*2+b at perm pos b*16+g*2+a2
                        dst = bass.AP(
                            tensor=sm.tensor, offset=sm.offset + b * 16,
                            ap=[list(sm.ap[0]), [2, 8], [1, 2]])
                    nc.vector.tensor_tensor(
                        dst,
                        ex.rearrange("p (g r) -> p g r", r=gsz)[:, :, b],
                        rr, ALU.mult)
                return sm

            ps_p = psp.tile([T, 128], f32, tag="ps")
            for i in range(4):
                nc.tensor.matmul(
                    ps_p, rp.rearrange("p (t a) -> p a t", a=S)[:, i, :],
                    wpt[:, i * 128:(i + 1) * 128],
                    start=(i == 0), stop=(i == 3))
            p_sm = softmax_rowgroups("p", ps_p, bp, T, 32, 4)

            ps_q = psp.tile([T, 128], f32, tag="ps")
            for i in range(4):
                nc.tensor.matmul(
                    ps_q, cp.rearrange("p (t b) -> p b t", b=S)[:, i, :],
                    wqt[:, i * 128:(i + 1) * 128],
                    start=(i == 0), stop=(i == 3))
            q_sm = softmax_rowgroups("q", ps_q, bq, T, 32, 4)

            ps_t = psp.tile([1, 32], f32, tag="ps")
            for i in range(2):
                nc.tensor.matmul(
                    ps_t, tp[:, i:i + 1], wtt[:, i * 32:(i + 1) * 32],
                    start=(i == 0), stop=(i == 1))
            t_sm = softmax_rowgroups("t", ps_t, bt, 1, 16, 2)

            # ---- round-trip through DRAM (sm already perm-laid-out) ----
            p_scr = dramp.tile([T, 128], f32, tag="p_scr")
            nc.gpsimd.dma_start(p_scr, p_sm)
            q_scr = dramp.tile([T, 128], f32, tag="q_scr")
            nc.gpsimd.dma_start(q_scr, q_sm)
            t_scr = dramp.tile([1, 32], f32, tag="t_scr")
            nc.gpsimd.dma_start(t_scr, t_sm)

            # P_call[(t',a',b'), (g,a)] = p_scr[t', a'*32 + g*4 + a]
            p_call = mpool.tile([128, 32], f32, tag="p_call")
            for tpr in range(8):
                in_ap = bass.AP(tensor=p_scr.tensor,
                                offset=p_scr.offset + tpr * 128,
                                ap=[[32, 4], [0, 4], [1, 32]])
                nc.gpsimd.dma_start(p_call[tpr * 16:(tpr + 1) * 16, :], in_ap)
            # Q_call[(t',a',b'), (g,b)] = q_scr[t', b'*32 + g*4 + b]
            q_call = mpool.tile([128, 32], f32, tag="q_call")
            for tpr in range(8):
                in_ap = bass.AP(tensor=q_scr.tensor,
                                offset=q_scr.offset + tpr * 128,
                                ap=[[0, 4], [32, 4], [1, 32]])
                nc.gpsimd.dma_start(q_call[tpr * 16:(tpr + 1) * 16, :], in_ap)
            # T_call[(t',a',b'), (g,a2)] = t_scr[b2(t')*16 + g*2 + a2]
            t_call = mpool.tile([128, 16], f32, tag="t_call")
            in_ap = bass.AP(tensor=t_scr.tensor, offset=t_scr.offset,
                            ap=[[16, 2], [0, 64], [1, 16]])
            nc.gpsimd.dma_start(t_call, in_ap)

            # ---- build M_g [tab', tab] bf16 ----
            m_tiles = []
            for g in range(8):
                pq = mpool.tile([128, 128], f32, tag=f"pq_{g % 2}",
                                name=f"pq_{g}")
                p_ap = bass.AP(tensor=p_call.tensor,
                               offset=p_call.offset + g * 4,
                               ap=[list(p_call.ap[0]), [0, 8], [1, 4],
                                   [0, 4]])
                q_ap = bass.AP(tensor=q_call.tensor,
                               offset=q_call.offset + g * 4,
                               ap=[list(q_call.ap[0]), [0, 8], [0, 4],
                                   [1, 4]])
                nc.vector.tensor_tensor(pq, p_ap, q_ap, ALU.mult)
                pqt = mpool.tile([128, 128], f32, tag=f"pqt_{g % 2}",
                                 name=f"pqt_{g}")
                t_ap = bass.AP(tensor=t_call.tensor,
                               offset=t_call.offset + g * 2,
                               ap=[list(t_call.ap[0]), [1, 2], [0, 4],
                                   [0, 16]])
                nc.vector.tensor_tensor(pqt, pq, t_ap, ALU.mult)
                mg = mpool.tile([128, 128], bf16, tag=f"mg_{g}")
                nc.gpsimd.tensor_tensor(mg, pqt, mask, ALU.mult)
                m_tiles.append(mg)

            # ================= mix + T-out + unshuffle ====================
            y2n = []
            for ch in range(NCHUNK):
                # mix: psum[tab, (uv8, c64)] = M_g.T? (lhsT=M_g) @ XT-slice
                yt_ch = shortp.tile([128, POS], bf16, tag="sh",
                                    name=f"yt_{ch}")
                for half in range(2):
                    g = ch * 2 + half
                    for blk in range(8):
                        mps = psp.tile([128, 512], f32, tag="ps")
                        rhs = bass.AP(
                            tensor=xt[ch].tensor,
                            offset=xt[ch].offset + blk * 1024 + half * 64,
                            ap=[list(xt[ch].ap[0]), [128, 8], [1, 64]])
                        nc.tensor.matmul(mps, m_tiles[g], rhs,
                                         start=True, stop=True)
                        out_ap = bass.AP(
                            tensor=yt_ch.tensor,
                            offset=yt_ch.offset + blk * 1024 + half * 64,
                            ap=[list(yt_ch.ap[0]), [128, 8], [1, 64]])
                        nc.scalar.activation(out_ap, mps, AF.Copy)
                # T-out: per o-quad: 4 transposes; copies write y2 L3 direct
                y2 = longp.tile([128, POS], bf16, tag="lg", name=f"y2_{ch}")
                for oct in range(8):
                    tps = psp.tile([128, 1024], bf16, tag="ps")
                    for j in range(8):
                        o = oct * 8 + j
                        in_ap = bass.AP(tensor=yt_ch.tensor,
                                        offset=yt_ch.offset + o * 128,
                                        ap=[list(yt_ch.ap[0]), [1, 128]])
                        nc.tensor.transpose(
                            tps[:, j * 128:(j + 1) * 128], in_ap, ident)
                    # psum [c, (j8, tab128)] -> y2[tab*64 + oct*8 + j]
                    in_ap = bass.AP(tensor=tps.tensor, offset=tps.offset,
                                    ap=[list(tps.ap[0]), [1, 128], [128, 8]])
                    out_ap = bass.AP(tensor=y2.tensor,
                                     offset=y2.offset + oct * 8,
                                     ap=[list(y2.ap[0]), [64, 128], [1, 8]])
                    nc.vector.tensor_copy(out_ap, in_ap)
                y2n.append(y2)

            # ================= conv2 + BN + ReLU -> y =====================
            for oc in range(NCHUNK):
                for ntg in range(4):
                    pss = [psa.tile([CPK, 512], f32, tag=f"c1ps_{j}",
                                    name=f"ps_c2_{oc}_{ntg}_{j}")
                           for j in range(4)]
                    for cq in range(NCHUNK):
                        for j in range(4):
                            nt = ntg * 4 + j
                            nc.tensor.matmul(
                                pss[j],
                                w2t[cq][:, oc * CPK:(oc + 1) * CPK],
                                y2n[cq][:, nt * 512:(nt + 1) * 512],
                                start=(cq == 0), stop=(cq == NCHUNK - 1))
                    for j in range(4):
                        nt = ntg * 4 + j
                        ob = outp.tile([CPK, 512], f32, tag="ob")
                        # psum is L3-ordered (ta2, b, v, u); scatter to
                        # natural (ta2, u, b, v): iter (ta2, b, v, u) ->
                        # out strides (256, 8, 1, 32); (b,v) merge @1 x32
                        out_ap = bass.AP(
                            tensor=ob.tensor, offset=ob.offset,
                            ap=[list(ob.ap[0]), [256, 2], [1, 32], [32, 8]])
                        if j % 2 == 0:
                            nc.scalar.activation(out_ap, pss[j], AF.Relu,
                                                 bias=b2[oc], scale=1.0)
                        else:
                            nc.vector.tensor_scalar(
                                out_ap, pss[j], b2[oc], 0.0,
                                ALU.add, ALU.max)
                        nc.sync.dma_start(
                            y_d[:][oc * CPK:(oc + 1) * CPK,
                                   nt * 512:(nt + 1) * 512], ob)

    _split_waits(nc, mybir)
    return nc


def _host_prep(inputs):
    """Fold BN into conv weights, build device-layout weight arrays."""
    f = np.float32
    conv1_w = np.asarray(inputs["conv1_w"], f)
    conv1_b = np.asarray(inputs["conv1_b"], f)
    s1 = np.asarray(inputs["bn1_g"], f) / np.sqrt(
        np.asarray(inputs["bn1_v"], f) + EPS)
    w1 = conv1_w * s1[:, None]
    b1 = (conv1_b - np.asarray(inputs["bn1_m"], f)) * s1 \
        + np.asarray(inputs["bn1_b"], f)

    convp_w = np.asarray(inputs["convp_w"], f)   # (128, 8, 4)
    convq_w = np.asarray(inputs["convq_w"], f)
    convt_w = np.asarray(inputs["convt_w"], f)   # (32, 8, 2)

    wpt = np.transpose(convp_w, (1, 2, 0)).reshape(K, 4 * 128).copy()
    bp = np.tile(np.asarray(inputs["convp_b"], f)[None, :], (T, 1))

    # permute q outputs o=k*16+a*4+b -> o'=k*16+b*4+a (a innermost)
    perm = np.arange(128).reshape(8, 4, 4).transpose(0, 2, 1).reshape(128)
    wq_p = convq_w[perm]
    bq_p = np.asarray(inputs["convq_b"], f)[perm]
    wqt = np.transpose(wq_p, (1, 2, 0)).reshape(K, 4 * 128).copy()
    bq = np.tile(bq_p[None, :], (T, 1))

    wtt = (np.transpose(convt_w, (1, 2, 0)) / (T // TS * H * W)).reshape(
        K, 2 * 32).copy()
    bt = np.asarray(inputs["convt_b"], f).reshape(1, 32)

    conv2_w = np.asarray(inputs["conv2_w"], f)
    s2 = np.asarray(inputs["bn2_g"], f) / np.sqrt(
        np.asarray(inputs["bn2_v"], f) + EPS)
    w2 = conv2_w * s2[:, None]
    b2 = (np.asarray(inputs["conv2_b"], f)
          - np.asarray(inputs["bn2_m"], f)) * s2 \
        + np.asarray(inputs["bn2_b"], f)

    # delta mask: mask[(t',a',b'), (t,a,b)] = (t' % 4 == t % 4)
    tb_in = (np.arange(128) // 16) % 4
    tb_out = (np.arange(128) // 16) % 4
    mask = (tb_in[:, None] == tb_out[None, :]).astype(f)

    bf = ml_dtypes.bfloat16
    return {
        "w1t": np.ascontiguousarray(w1.T).astype(bf),
        "b1": b1.reshape(K, 1).copy(),
        "wpt": wpt, "bp": bp.copy(),
        "wqt": wqt, "bq": bq.copy(),
        "wtt": wtt, "bt": bt.copy(),
        "w2t": np.ascontiguousarray(w2.T).astype(bf),
        "b2": b2.reshape(C, 1).copy(),
        "mask": mask,
    }


def kernel(**inputs) -> np.ndarray:
    from concourse.bass_utils import run_bass_kernel_spmd

    if "nc" not in _CACHE:
        _CACHE["nc"] = _build()
    nc = _CACHE["nc"]

    shared = _host_prep(inputs)
    x = np.asarray(inputs["x"], np.float32)       # (8, 512, 8, 32, 32)
    in_maps = []
    for i in range(NCORES):
        m = dict(shared)
        m["x"] = np.ascontiguousarray(x[i].reshape(C, POS))
        in_maps.append(m)

    res = run_bass_kernel_spmd(nc, in_maps, list(range(NCORES)))
    out = np.stack([res.results[i]["y"].reshape(C, T, H, W)
                    for i in range(NCORES)])
    return out.astype(np.float32)


# revision 22
# speedup vs baseline: 1.0993x; 1.0993x over previous
"""Trainium2 Bass kernel for nn_BATransform — PE-matmul spatial mix.

Per core (one sample, c-major [C, 8192] f32 in DRAM):
  load+convert+shuffle x -> L-layout (u,v,t,a,b) bf16 chunks
  conv1(1x1)+BN+ReLU -> o (k=8) in L-layout -> pools -> p/q/tm softmax
  build per-group 128x128 mix matrices M_g = TM (x) P (x) Q on DVE
  T-in: PE transposes -> XT[(t,a,b), (u,v,c)]
  mix: 8 matmuls/chunk with M_g -> YT -> T-out: PE transposes -> y2L
  unshuffle (2-pass) -> y2 natural c-major -> conv2+BN+ReLU -> store
"""

import numpy as np
import ml_dtypes

NB, C, T, H, W = 8, 512, 8, 32, 32
S, TS, K, TK = 4, 2, 8, 8
EPS = 1e-5
POS = T * H * W        # 8192
NCORES = 8
NCHUNK = 4
CPK = 128

_CACHE = {}


def _split_waits(nc, mybir, max_waits=1):
    """Walrus accepts at most one sync-wait per instruction; hoist extras
    onto same-engine NoOps placed just before."""
    for f in nc.m.functions:
        for bb in f.blocks:
            out = []
            for inst in bb.instructions:
                si = inst.sync_info
                if si is not None and si.on_wait and len(si.on_wait) > max_waits:
                    waits = list(si.on_wait)
                    keep = waits[-max_waits:]
                    extra = waits[:-max_waits]
                    k = 0
                    while extra:
                        chunk, extra = extra[:max_waits], extra[max_waits:]
                        nop = mybir.InstNoOp(
                            name=f"{inst.name}-ws{k}",
                            sync_info=mybir.SyncInfo(on_wait=chunk, on_update=[]),
                        )
                        nop.engine = inst.engine
                        nc.register_instruction(nop)
                        out.append(nop)
                        k += 1
                    inst.sync_info = mybir.SyncInfo(
                        on_wait=keep, on_update=list(si.on_update)
                    )
                out.append(inst)
            bb.instructions = out


def _build():
    import concourse.bass as bass
    import concourse.tile as tile
    import concourse.mybir as mybir
    from concourse.masks import make_identity

    f32 = mybir.dt.float32
    bf16 = mybir.dt.bfloat16
    AF = mybir.ActivationFunctionType
    ALU = mybir.AluOpType
    AX = mybir.AxisListType

    nc = bass.Bass("TRN2", target_bir_lowering=False, debug=False,
                   num_devices=NCORES)

    x_d = nc.dram_tensor("x", [C, POS], f32, kind="ExternalInput")
    w1t_d = nc.dram_tensor("w1t", [C, K], bf16, kind="ExternalInput")
    b1_d = nc.dram_tensor("b1", [K, 1], f32, kind="ExternalInput")
    wpt_d = nc.dram_tensor("wpt", [K, 4 * 128], f32, kind="ExternalInput")
    bp_d = nc.dram_tensor("bp", [T, 128], f32, kind="ExternalInput")
    wqt_d = nc.dram_tensor("wqt", [K, 4 * 128], f32, kind="ExternalInput")
    bq_d = nc.dram_tensor("bq", [T, 128], f32, kind="ExternalInput")
    wtt_d = nc.dram_tensor("wtt", [K, 2 * 32], f32, kind="ExternalInput")
    bt_d = nc.dram_tensor("bt", [1, 32], f32, kind="ExternalInput")
    w2t_d = nc.dram_tensor("w2t", [C, C], bf16, kind="ExternalInput")
    b2_d = nc.dram_tensor("b2", [C, 1], f32, kind="ExternalInput")
    mask_d = nc.dram_tensor("mask", [128, 128], f32, kind="ExternalInput")
    y_d = nc.dram_tensor("y", [C, POS], f32, kind="ExternalOutput")

    with tile.TileContext(nc) as tc:
        with (
            tc.tile_pool(name="consts", bufs=1) as consts,
            tc.tile_pool(name="stage", bufs=2) as stage,      # f32 [128,2048]
            tc.tile_pool(name="short", bufs=3) as shortp,     # bf16 [128,8192]
            tc.tile_pool(name="long", bufs=4) as longp,       # XT + y2n shared
            tc.tile_pool(name="osb", bufs=1) as osbp,
            tc.tile_pool(name="sm2", bufs=1) as sm2,
            tc.tile_pool(name="mpool", bufs=1) as mpool,
            tc.tile_pool(name="outp", bufs=4) as outp,        # f32 [128,512]
            tc.tile_pool(name="psA", bufs=1, space="PSUM") as psa,
            tc.tile_pool(name="psB", bufs=4, space="PSUM") as psp,
            tc.tile_pool(name="dram", bufs=1, space="DRAM") as dramp,
        ):
            # ---- weights / consts ----
            w1t = []
            for q in range(NCHUNK):
                t_ = consts.tile([CPK, K], bf16, tag=f"w1t{q}")
                nc.sync.dma_start(t_, w1t_d[:][q * CPK:(q + 1) * CPK, :])
                w1t.append(t_)
            b1 = consts.tile([K, 1], f32, tag="b1")
            nc.sync.dma_start(b1, b1_d[:])
            wpt = consts.tile([K, 4 * 128], f32, tag="wpt")
            nc.sync.dma_start(wpt, wpt_d[:])
            bp = consts.tile([T, 128], f32, tag="bp")
            nc.sync.dma_start(bp, bp_d[:])
            wqt = consts.tile([K, 4 * 128], f32, tag="wqt")
            nc.sync.dma_start(wqt, wqt_d[:])
            bq = consts.tile([T, 128], f32, tag="bq")
            nc.sync.dma_start(bq, bq_d[:])
            wtt = consts.tile([K, 2 * 32], f32, tag="wtt")
            nc.sync.dma_start(wtt, wtt_d[:])
            bt = consts.tile([1, 32], f32, tag="bt")
            nc.sync.dma_start(bt, bt_d[:])
            w2t = []
            for q in range(NCHUNK):
                t_ = consts.tile([CPK, C], bf16, tag=f"w2t{q}")
                nc.sync.dma_start(t_, w2t_d[:][q * CPK:(q + 1) * CPK, :])
                w2t.append(t_)
            b2 = []
            for oc in range(NCHUNK):
                t_ = consts.tile([CPK, 1], f32, tag=f"b2_{oc}")
                nc.sync.dma_start(t_, b2_d[:][oc * CPK:(oc + 1) * CPK, :])
                b2.append(t_)
            mask = consts.tile([128, 128], f32, tag="mask")
            nc.sync.dma_start(mask, mask_d[:])
            ident = consts.tile([128, 128], bf16, tag="ident")
            make_identity(nc, ident)

            o_sb = osbp.tile([K, POS], bf16, tag="o")
            # conv1 accumulators: 4 psum banks x 4 partition offsets (32j)
            c1ps = [psa.tile([128, 512], f32, tag=f"c1ps_{i}",
                             name=f"c1ps_{i}")
                    for i in range(4)]

            xt = []     # XT chunks [tab, (o,c)]
            # ================= load + shuffle + conv1-partial + T-in ======
            for ch in range(NCHUNK):
                # fused cast+Lshuf: natural f32 quarter -> L3 bf16 quarter
                # iter (ta 8, bv 32, u 8): in strides (256, 1, 32)
                x2 = shortp.tile([CPK, POS], bf16, tag="sh", name=f"x2_{ch}")
                for q in range(4):
                    xf = stage.tile([CPK, POS // 4], f32, tag="stage")
                    nc.sync.dma_start(
                        xf, x_d[:][ch * CPK:(ch + 1) * CPK,
                                   q * 2048:(q + 1) * 2048])
                    in_ap = bass.AP(tensor=xf.tensor, offset=xf.offset,
                                    ap=[list(xf.ap[0]), [256, 8], [1, 32],
                                        [32, 8]])
                    if q % 2 == 0:
                        nc.scalar.activation(
                            x2[:, q * 2048:(q + 1) * 2048], in_ap, AF.Copy)
                    else:
                        nc.vector.tensor_copy(
                            x2[:, q * 2048:(q + 1) * 2048], in_ap)

                # conv1 partials: psum[nt] += w1t[ch].T @ x2[:, nt*512...]
                for nt in range(16):
                    ps = c1ps[nt % 4][(nt // 4) * 32:(nt // 4) * 32 + K, :]
                    nc.tensor.matmul(
                        ps, w1t[ch], x2[:, nt * 512:(nt + 1) * 512],
                        start=(ch == 0), stop=(ch == NCHUNK - 1),
                        tile_position=(0, (nt // 4) * 32))

                # T-in: per uv-quad: 4 transposes + 1 copy
                xt_ch = longp.tile([128, POS], bf16, tag="lg",
                                   name=f"xt_{ch}")
                for oct in range(8):
                    tps = psp.tile([128, 1024], bf16, tag="ps")
                    for j in range(8):
                        o = oct * 8 + j
                        in_ap = bass.AP(tensor=x2.tensor,
                                        offset=x2.offset + o,
                                        ap=[list(x2.ap[0]), [64, 128]])
                        nc.tensor.transpose(
                            tps[:, j * 128:(j + 1) * 128], in_ap, ident)
                    nc.vector.tensor_copy(
                        xt_ch[:, oct * 1024:(oct + 1) * 1024], tps)
                xt.append(xt_ch)

            # ---- conv1 finalize: ReLU(bias) -> o_sb; block sums -> tpp --
            tpp = sm2.tile([K, 16], f32, tag="tpp")
            for nt in range(16):
                ps = c1ps[nt % 4][(nt // 4) * 32:(nt // 4) * 32 + K, :]
                nc.scalar.activation(
                    o_sb[:, nt * 512:(nt + 1) * 512], ps, AF.Relu,
                    bias=b1, scale=1.0, accum_out=tpp[:, nt:nt + 1])

            # ---- pools (o in L3 layout (t,a,b,v,u)) ----
            # rp[k, (t,a)] = max over (b, v, u) = inner 256
            rp = sm2.tile([K, T * S], f32, tag="rp")
            nc.vector.tensor_reduce(
                rp, bass.AP(tensor=o_sb.tensor, offset=o_sb.offset,
                            ap=[list(o_sb.ap[0]), [256, 32], [1, 256]]),
                AX.X, ALU.max)
            # cp[k, (t,b)] = max over (a, (v,u) 64)
            cp = sm2.tile([K, T * S], f32, tag="cp")
            nc.vector.tensor_reduce(
                cp, bass.AP(tensor=o_sb.tensor, offset=o_sb.offset,
                            ap=[list(o_sb.ap[0]), [1024, 8], [64, 4],
                                [256, 4], [1, 64]]),
                AX.XY, ALU.max)
            # tp[k, ts] = sum of tpp block sums
            tp = sm2.tile([K, TS], f32, tag="tp")
            nc.vector.tensor_reduce(
                tp, tpp.rearrange("p (s j) -> p s j", j=8), AX.X, ALU.add)

            # ---- p/q/tm small matmuls + softmax ----
            def softmax_rowgroups(name, ps_tile, bias_tile, parts, ngroups,
                                  gsz):
                raw = sm2.tile([parts, ngroups * gsz], f32, tag=f"{name}_raw")
                nc.vector.tensor_tensor(raw, ps_tile, bias_tile, ALU.add)
                ex = sm2.tile([parts, ngroups * gsz], f32, tag=f"{name}_ex")
                nc.scalar.activation(ex, raw, AF.Exp)
                ss = sm2.tile([parts, ngroups], f32, tag=f"{name}_ss")
                nc.vector.tensor_reduce(
                    ss, ex.rearrange("p (g r) -> p g r", r=gsz), AX.X,
                    ALU.add)
                rr = sm2.tile([parts, ngroups], f32, tag=f"{name}_rr")
                nc.vector.reciprocal(rr, ss)
                sm = sm2.tile([parts, ngroups * gsz], f32, tag=f"{name}_sm")
                for b in range(gsz):
                    if name in ("p", "q"):
                        # write col g*16+a*4+b at perm pos b*32+g*4+a
                        dst = bass.AP(
                            tensor=sm.tensor, offset=sm.offset + b * 32,
                            ap=[list(sm.ap[0]), [4, 8], [1, 4]])
                    else:
                        # t: col g*4+a2*2+b at perm pos b*16+g*2+a2
                        dst = bass.AP(
                            tensor=sm.tensor, offset=sm.offset + b * 16,
                            ap=[list(sm.ap[0]), [2, 8], [1, 2]])
                    nc.vector.tensor_tensor(
                        dst,
                        ex.rearrange("p (g r) -> p g r", r=gsz)[:, :, b],
                        rr, ALU.mult)
                return sm

            ps_p = psp.tile([T, 128], f32, tag="ps")
            for i in range(4):
                nc.tensor.matmul(
                    ps_p, rp.rearrange("p (t a) -> p a t", a=S)[:, i, :],
                    wpt[:, i * 128:(i + 1) * 128],
                    start=(i == 0), stop=(i == 3))
            p_sm = softmax_rowgroups("p", ps_p, bp, T, 32, 4)

            ps_q = psp.tile([T, 128], f32, tag="ps")
            for i in range(4):
                nc.tensor.matmul(
                    ps_q, cp.rearrange("p (t b) -> p b t", b=S)[:, i, :],
                    wqt[:, i * 128:(i + 1) * 128],
                    start=(i == 0), stop=(i == 3))
            q_sm = softmax_rowgroups("q", ps_q, bq, T, 32, 4)

            ps_t = psp.tile([1, 32], f32, tag="ps")
            for i in range(2):
                nc.tensor.matmul(
                    ps_t, tp[:, i:i + 1], wtt[:, i * 32:(i + 1) * 32],
                    start=(i == 0), stop=(i == 1))
            t_sm = softmax_rowgroups("t", ps_t, bt, 1, 16, 2)

            # ---- round-trip through DRAM (sm already perm-laid-out) ----
            p_scr = dramp.tile([T, 128], f32, tag="p_scr")
            nc.gpsimd.dma_start(p_scr, p_sm)
            q_scr = dramp.tile([T, 128], f32, tag="q_scr")
            nc.gpsimd.dma_start(q_scr, q_sm)
            t_scr = dramp.tile([1, 32], f32, tag="t_scr")
            nc.gpsimd.dma_start(t_scr, t_sm)

            # P_call[(t',a',b'), (g,a)] = p_scr[t', a'*32 + g*4 + a]
            p_call = mpool.tile([128, 32], f32, tag="p_call")
            for tpr in range(8):
                in_ap = bass.AP(tensor=p_scr.tensor,
                                offset=p_scr.offset + tpr * 128,
                                ap=[[32, 4], [0, 4], [1, 32]])
                nc.gpsimd.dma_start(p_call[tpr * 16:(tpr + 1) * 16, :], in_ap)
            # Q_call[(t',a',b'), (g,b)] = q_scr[t', b'*32 + g*4 + b]
            q_call = mpool.tile([128, 32], f32, tag="q_call")
            for tpr in range(8):
                in_ap = bass.AP(tensor=q_scr.tensor,
                                offset=q_scr.offset + tpr * 128,
                                ap=[[0, 4], [32, 4], [1, 32]])
                nc.gpsimd.dma_start(q_call[tpr * 16:(tpr + 1) * 16, :], in_ap)
            # T_call[(t',a',b'), (g,a2)] = t_scr[b2(t')*16 + g*2 + a2]
            t_call = mpool.tile([128, 16], f32, tag="t_call")
            in_ap = bass.AP(tensor=t_scr.tensor, offset=t_scr.offset,
                            ap=[[16, 2], [0, 64], [1, 16]])
            nc.gpsimd.dma_start(t_call, in_ap)

            # ---- build M_g [tab', tab] bf16 ----
            m_tiles = []
            for g in range(8):
                pq = mpool.tile([128, 128], f32, tag=f"pq_{g % 2}",
                                name=f"pq_{g}")
                p_ap = bass.AP(tensor=p_call.tensor,
                               offset=p_call.offset + g * 4,
                               ap=[list(p_call.ap[0]), [0, 8], [1, 4],
                                   [0, 4]])
                q_ap = bass.AP(tensor=q_call.tensor,
                               offset=q_call.offset + g * 4,
                               ap=[list(q_call.ap[0]), [0, 8], [0, 4],
                                   [1, 4]])
                nc.vector.tensor_tensor(pq, p_ap, q_ap, ALU.mult)
                pqt = mpool.tile([128, 128], f32, tag=f"pqt_{g % 2}",
                                 name=f"pqt_{g}")
                t_ap = bass.AP(tensor=t_call.tensor,
                               offset=t_call.offset + g * 2,
                               ap=[list(t_call.ap[0]), [1, 2], [0, 4],
                                   [0, 16]])
                nc.vector.tensor_tensor(pqt, pq, t_ap, ALU.mult)
                mg = mpool.tile([128, 128], bf16, tag=f"mg_{g}")
                nc.gpsimd.tensor_tensor(mg, pqt, mask, ALU.mult)
                m_tiles.append(mg)

            # ================= mix + T-out + unshuffle ====================
            y2n = []
            for ch in range(NCHUNK):
                # mix: psum[tab, (uv8, c64)] = M_g.T? (lhsT=M_g) @ XT-slice
                yt_ch = shortp.tile([128, POS], bf16, tag="sh",
                                    name=f"yt_{ch}")
                for half in range(2):
                    g = ch * 2 + half
                    for blk in range(8):
                        mps = psp.tile([128, 512], f32, tag="ps")
                        rhs = bass.AP(
                            tensor=xt[ch].tensor,
                            offset=xt[ch].offset + blk * 1024 + half * 64,
                            ap=[list(xt[ch].ap[0]), [128, 8], [1, 64]])
                        nc.tensor.matmul(mps, m_tiles[g], rhs,
                                         start=True, stop=True)
                        out_ap = bass.AP(
                            tensor=yt_ch.tensor,
                            offset=yt_ch.offset + blk * 1024 + half * 64,
                            ap=[list(yt_ch.ap[0]), [128, 8], [1, 64]])
                        nc.scalar.activation(out_ap, mps, AF.Copy)
                # T-out: per o-quad: 4 transposes; copies write y2 L3 direct
                y2 = longp.tile([128, POS], bf16, tag="lg", name=f"y2_{ch}")
                for oct in range(8):
                    tps = psp.tile([128, 1024], bf16, tag="ps")
                    for j in range(8):
                        o = oct * 8 + j
                        in_ap = bass.AP(tensor=yt_ch.tensor,
                                        offset=yt_ch.offset + o * 128,
                                        ap=[list(yt_ch.ap[0]), [1, 128]])
                        nc.tensor.transpose(
                            tps[:, j * 128:(j + 1) * 128], in_ap, ident)
                    # psum [c, (j8, tab128)] -> y2[tab*64 + oct*8 + j]
                    in_ap = bass.AP(tensor=tps.tensor, offset=tps.offset,
                                    ap=[list(tps.ap[0]), [1, 128], [128, 8]])
                    out_ap = bass.AP(tensor=y2.tensor,
                                     offset=y2.offset + oct * 8,
                                     ap=[list(y2.ap[0]), [64, 128], [1, 8]])
                    nc.vector.tensor_copy(out_ap, in_ap)
                y2n.append(y2)

            # ================= conv2 + BN + ReLU -> y =====================
            for oc in range(NCHUNK):
                for ntg in range(4):
                    pss = [psa.tile([CPK, 512], f32, tag=f"c1ps_{j}",
                                    name=f"ps_c2_{oc}_{ntg}_{j}")
                           for j in range(4)]
                    for cq in range(NCHUNK):
                        for j in range(4):
                            nt = ntg * 4 + j
                            nc.tensor.matmul(
                                pss[j],
                                w2t[cq][:, oc * CPK:(oc + 1) * CPK],
                                y2n[cq][:, nt * 512:(nt + 1) * 512],
                                start=(cq == 0), stop=(cq == NCHUNK - 1))
                    for j in range(4):
                        nt = ntg * 4 + j
                        ob = outp.tile([CPK, 512], f32, tag="ob")
                        # psum is L3-ordered (ta2, b, v, u); scatter to
                        # natural (ta2, u, b, v): iter (ta2, b, v, u) ->
                        # out strides (256, 8, 1, 32); (b,v) merge @1 x32
                        out_ap = bass.AP(
                            tensor=ob.tensor, offset=ob.offset,
                            ap=[list(ob.ap[0]), [256, 2], [1, 32], [32, 8]])
                        if j % 2 == 0:
                            nc.scalar.activation(out_ap, pss[j], AF.Relu,
                                                 bias=b2[oc], scale=1.0)
                        else:
                            nc.vector.tensor_scalar(
                                out_ap, pss[j], b2[oc], 0.0,
                                ALU.add, ALU.max)
                        nc.sync.dma_start(
                            y_d[:][oc * CPK:(oc + 1) * CPK,
                                   nt * 512:(nt + 1) * 512], ob)

    _split_waits(nc, mybir)
    return nc


def _host_prep(inputs):
    """Fold BN into conv weights, build device-layout weight arrays."""
    f = np.float32
    conv1_w = np.asarray(inputs["conv1_w"], f)
    conv1_b = np.asarray(inputs["conv1_b"], f)
    s1 = np.asarray(inputs["bn1_g"], f) / np.sqrt(
        np.asarray(inputs["bn1_v"], f) + EPS)
    w1 = conv1_w * s1[:, None]
    b1 = (conv1_b - np.asarray(inputs["bn1_m"], f)) * s1 \
        + np.asarray(inputs["bn1_b"], f)

    convp_w = np.asarray(inputs["convp_w"], f)   # (128, 8, 4)
    convq_w = np.asarray(inputs["convq_w"], f)
    convt_w = np.asarray(inputs["convt_w"], f)   # (32, 8, 2)

    wpt = np.transpose(convp_w, (1, 2, 0)).reshape(K, 4 * 128).copy()
    bp = np.tile(np.asarray(inputs["convp_b"], f)[None, :], (T, 1))

    # permute q outputs o=k*16+a*4+b -> o'=k*16+b*4+a (a innermost)
    perm = np.arange(128).reshape(8, 4, 4).transpose(0, 2, 1).reshape(128)
    wq_p = convq_w[perm]
    bq_p = np.asarray(inputs["convq_b"], f)[perm]
    wqt = np.transpose(wq_p, (1, 2, 0)).reshape(K, 4 * 128).copy()
    bq = np.tile(bq_p[None, :], (T, 1))

    wtt = (np.transpose(convt_w, (1, 2, 0)) / (T // TS * H * W)).reshape(
        K, 2 * 32).copy()
    bt = np.asarray(inputs["convt_b"], f).reshape(1, 32)

    conv2_w = np.asarray(inputs["conv2_w"], f)
    s2 = np.asarray(inputs["bn2_g"], f) / np.sqrt(
        np.asarray(inputs["bn2_v"], f) + EPS)
    w2 = conv2_w * s2[:, None]
    b2 = (np.asarray(inputs["conv2_b"], f)
          - np.asarray(inputs["bn2_m"], f)) * s2 \
        + np.asarray(inputs["bn2_b"], f)

    # delta mask: mask[(t',a',b'), (t,a,b)] = (t' % 4 == t % 4)
    tb_in = (np.arange(128) // 16) % 4
    tb_out = (np.arange(128) // 16) % 4
    mask = (tb_in[:, None] == tb_out[None, :]).astype(f)

    bf = ml_dtypes.bfloat16
    return {
        "w1t": np.ascontiguousarray(w1.T).astype(bf),
        "b1": b1.reshape(K, 1).copy(),
        "wpt": wpt, "bp": bp.copy(),
        "wqt": wqt, "bq": bq.copy(),
        "wtt": wtt, "bt": bt.copy(),
        "w2t": np.ascontiguousarray(w2.T).astype(bf),
        "b2": b2.reshape(C, 1).copy(),
        "mask": mask,
    }


def kernel(**inputs) -> np.ndarray:
    from concourse.bass_utils import run_bass_kernel_spmd

    if "nc" not in _CACHE:
        _CACHE["nc"] = _build()
    nc = _CACHE["nc"]

    shared = _host_prep(inputs)
    x = np.asarray(inputs["x"], np.float32)       # (8, 512, 8, 32, 32)
    in_maps = []
    for i in range(NCORES):
        m = dict(shared)
        m["x"] = np.ascontiguousarray(x[i].reshape(C, POS))
        in_maps.append(m)

    res = run_bass_kernel_spmd(nc, in_maps, list(range(NCORES)))
    out = np.stack([res.results[i]["y"].reshape(C, T, H, W)
                    for i in range(NCORES)])
    return out.astype(np.float32)
